# revision 4
# baseline (speedup 1.0000x reference)
"""CGMM layer (segment_reduce) Trainium2 kernel.

Math: every per-node quantity depends on the node only through its discrete
label x_n (64 values), so the kernel reduces to:
  1. a tiny 64x24 table build from B/Pi on device (softmaxes, posterior,
     max/argmax over C, per-label likelihood),
  2. a table gather for all N nodes done on the tensor engine: one-hot(x)
     built by DVE is_equal against a per-partition label column, then a
     block-diagonal matmul (4 node-slots per column, 24 outputs each),
  3. an unsegmented per-chunk prefix sum (DVE tensor_tensor_scan) of the
     per-node likelihood; the host samples it at graph boundaries (known
     from the sorted `batch`) and differences to get per-graph sums.

Sharding: data-parallel over nodes, 37500 nodes per core across 8 cores;
B/Pi replicated. Graph segment sums are stitched on the host from per-chunk
prefix sums (a graph can straddle chunk/core boundaries).
"""

import numpy as np
import ml_dtypes
from contextlib import ExitStack

import concourse.bass as bass
import concourse.tile as tile
import concourse.mybir as mybir
from concourse import bacc
from concourse.bass_utils import run_bass_kernel_spmd

# problem constants (hardcoded per harness contract)
N = 300000
C = 32
M = 64
NGEN = 8
G = 8192
NCORES = 8
NPC = N // NCORES          # 37500 nodes per core
SLOT = 9376                # nodes per slot (4 slots per core, 4 pad nodes)
NSLOT = 4
CHUNK = 2344               # scan chunk (SLOT = 4 * CHUNK)
NCHUNK = 16                # chunks per core (16 * 2344 = 37504)
BIG = 10000.0

_cache = {}


def _build_nc():
    nc = bacc.Bacc("TRN2", target_bir_lowering=False, debug=False,
                   num_devices=NCORES)
    f32, bf16 = mybir.dt.float32, mybir.dt.bfloat16

    Bp = nc.declare_dram_parameter("Bp", [C, M * NGEN], f32, isOutput=False)
    Pip = nc.declare_dram_parameter("Pip", [C, NGEN], f32, isOutput=False)
    xb_in = nc.declare_dram_parameter("xb", [128, SLOT], bf16, isOutput=False)
    ilo_in = nc.declare_dram_parameter("ilo", [128, 1], f32, isOutput=False)
    ihi_in = nc.declare_dram_parameter("ihi", [128, 1], f32, isOutput=False)
    iotac_in = nc.declare_dram_parameter("iotac", [C, M * NGEN], f32, isOutput=False)

    hv_out = nc.declare_dram_parameter("hv", [NSLOT, SLOT, NGEN], f32, isOutput=True)
    hi_out = nc.declare_dram_parameter("hi", [NSLOT, SLOT, NGEN], f32, isOutput=True)
    lc_out = nc.declare_dram_parameter("lc", [128, CHUNK], f32, isOutput=True)

    with tile.TileContext(nc, num_cores=NCORES) as tc:
        with ExitStack() as ctx:
            small = ctx.enter_context(tc.tile_pool(name="small", bufs=1))
            big = ctx.enter_context(tc.tile_pool(name="big", bufs=1))
            psum = ctx.enter_context(tc.tile_pool(name="ps", bufs=4, space="PSUM"))
            dpool = ctx.enter_context(tc.tile_pool(name="dscratch", bufs=1, space="DRAM"))
            # DRAM scratch for table layout shuffles
            dW = dpool.tile([M, 24], f32)

            # ---------------- inputs ----------------
            xb = big.tile([128, SLOT], bf16)
            nc.sync.dma_start(xb[:], xb_in[:])
            ilo = small.tile([128, 1], f32)
            nc.sync.dma_start(ilo[:], ilo_in[:])
            ihi = small.tile([128, 1], f32)
            nc.sync.dma_start(ihi[:], ihi_in[:])
            iotac = small.tile([C, 512], f32)
            nc.sync.dma_start(iotac[:], iotac_in[:])
            Bt = small.tile([C, 512], f32)
            nc.sync.dma_start(Bt[:], Bp[:])
            Pit = small.tile([C, NGEN], f32)
            nc.sync.dma_start(Pit[:], Pip[:])

            # ---------------- table build (all tiny) ----------------
            # B softmax over m, per (c, g)
            eB = small.tile([C, 512], f32)
            nc.scalar.activation(eB[:], Bt[:], mybir.ActivationFunctionType.Exp)
            sB = small.tile([C, NGEN], f32)
            nc.vector.tensor_reduce(
                sB[:], eB[:].rearrange("p (m g) -> p g m", g=NGEN),
                axis=mybir.AxisListType.X, op=mybir.AluOpType.add)
            rB = small.tile([C, NGEN], f32)
            nc.vector.reciprocal(rB[:], sB[:])
            smB = small.tile([C, 512], f32)
            nc.vector.tensor_tensor(
                smB[:].rearrange("p (m g) -> p m g", g=NGEN),
                eB[:].rearrange("p (m g) -> p m g", g=NGEN),
                rB[:].unsqueeze(1).broadcast_to((C, M, NGEN)),
                op=mybir.AluOpType.mult)

            # Pi softmax over c (transpose to put c on the free axis)
            ePi = small.tile([C, NGEN], f32)
            nc.scalar.activation(ePi[:], Pit[:], mybir.ActivationFunctionType.Exp)
            piP = small.tile([C, 32], f32)
            nc.vector.memset(piP[:], 0.0)
            nc.vector.tensor_copy(piP[:, 0:NGEN], ePi[:])
            piT = small.tile([C, 32], f32)
            nc.vector.transpose(piT[:], piP[:])          # piT[g, c] (rows >=8 junk)
            sPi = small.tile([C, 1], f32)
            nc.vector.tensor_reduce(sPi[:], piT[:], axis=mybir.AxisListType.X,
                                    op=mybir.AluOpType.add)
            rPi = small.tile([C, 1], f32)
            nc.vector.reciprocal(rPi[:], sPi[:])
            smPiT = small.tile([C, 32], f32)
            nc.vector.tensor_scalar(smPiT[:], piT[:], rPi[:, 0:1], None,
                                    op0=mybir.AluOpType.mult)
            smPi32 = small.tile([C, 32], f32)
            nc.vector.transpose(smPi32[:], smPiT[:])     # smPi32[c, g] (cols >=8 junk)

            # numerator[c, m, g] = smB * smPi
            num = small.tile([C, 512], f32)
            nc.vector.tensor_tensor(
                num[:].rearrange("p (m g) -> p m g", g=NGEN),
                smB[:].rearrange("p (m g) -> p m g", g=NGEN),
                smPi32[:, 0:NGEN].unsqueeze(1).broadcast_to((C, M, NGEN)),
                op=mybir.AluOpType.mult)

            # transpose to [mg-within-block, c] blocks for c-reductions
            numT = small.tile([C, 512], f32)
            nc.vector.transpose(numT[:], num[:])
            den = small.tile([C, 16], f32)
            nc.vector.tensor_reduce(
                den[:], numT[:].rearrange("p (k q) -> p k q", q=32),
                axis=mybir.AxisListType.X, op=mybir.AluOpType.add)
            rden = small.tile([C, 16], f32)
            nc.vector.reciprocal(rden[:], den[:])
            postT = small.tile([C, 512], f32)
            nc.vector.tensor_tensor(
                postT[:].rearrange("p (k q) -> p k q", q=32),
                numT[:].rearrange("p (k q) -> p k q", q=32),
                rden[:].unsqueeze(2).broadcast_to((C, 16, 32)),
                op=mybir.AluOpType.mult)
            lognT = small.tile([C, 512], f32)
            nc.scalar.activation(lognT[:], numT[:], mybir.ActivationFunctionType.Ln)
            plT = small.tile([C, 512], f32)
            nc.vector.tensor_tensor(plT[:], postT[:], lognT[:],
                                    op=mybir.AluOpType.mult)
            likmg = small.tile([C, 16], f32)
            nc.vector.tensor_reduce(
                likmg[:], plT[:].rearrange("p (k q) -> p k q", q=32),
                axis=mybir.AxisListType.X, op=mybir.AluOpType.add)
            hvmg = small.tile([C, 16], f32)
            nc.vector.tensor_reduce(
                hvmg[:], postT[:].rearrange("p (k q) -> p k q", q=32),
                axis=mybir.AxisListType.X, op=mybir.AluOpType.max)
            mask = small.tile([C, 512], f32)
            nc.vector.tensor_tensor(
                mask[:].rearrange("p (k q) -> p k q", q=32),
                postT[:].rearrange("p (k q) -> p k q", q=32),
                hvmg[:].unsqueeze(2).broadcast_to((C, 16, 32)),
                op=mybir.AluOpType.is_equal)
            cand = small.tile([C, 512], f32)
            nc.vector.tensor_scalar(cand[:], mask[:], -BIG, None,
                                    op0=mybir.AluOpType.mult)
            cand2 = small.tile([C, 512], f32)
            nc.vector.tensor_tensor(cand2[:], cand[:], iotac[:],
                                    op=mybir.AluOpType.add)
            himg = small.tile([C, 16], f32)
            nc.vector.tensor_reduce(
                himg[:], cand2[:].rearrange("p (k q) -> p k q", q=32),
                axis=mybir.AxisListType.X, op=mybir.AluOpType.min)

            # dump tables into dW[m, 8t+g]: tile element (p=8a+b, k) holds
            # table value for mg = 32k + p, i.e. m = 4k + a, g = b
            dWv = dW[:, :].rearrange("(k a) (t b) -> a t b k", a=4, b=8)
            for t, tmg in enumerate((hvmg, himg, likmg)):
                for a in range(4):
                    nc.sync.dma_start(dWv[a, t], tmg[8 * a:8 * a + 8, :])

            # block-diagonal weights [128, 96]
            Wlo = small.tile([128, 96], f32)
            nc.vector.memset(Wlo[:], 0.0)
            Whi = small.tile([128, 96], f32)
            nc.vector.memset(Whi[:], 0.0)
            for s in range(NSLOT):
                nc.sync.dma_start(Wlo[32 * s:32 * s + 32, 24 * s:24 * s + 24],
                                  dW[0:32, :])
                nc.sync.dma_start(Whi[32 * s:32 * s + 32, 24 * s:24 * s + 24],
                                  dW[32:64, :])

            # ---------------- one-hot + gather matmuls ----------------
            oh_lo = big.tile([128, SLOT], f32)
            nc.vector.tensor_scalar(oh_lo[:], xb[:], ilo[:, 0:1], None,
                                    op0=mybir.AluOpType.is_equal)
            oh_hi = big.tile([128, SLOT], f32)
            nc.vector.tensor_scalar(oh_hi[:], xb[:], ihi[:, 0:1], None,
                                    op0=mybir.AluOpType.is_equal)

            gath = big.tile([96, SLOT], f32)
            nmm = 512
            off = 0
            while off < SLOT:
                n = min(nmm, SLOT - off)
                ps = psum.tile([96, n], f32)
                nc.tensor.matmul(ps[:], Wlo[:], oh_lo[:, off:off + n],
                                 start=True, stop=False)
                nc.tensor.matmul(ps[:], Whi[:], oh_hi[:, off:off + n],
                                 start=False, stop=True)
                nc.vector.tensor_copy(gath[0:64, off:off + n], ps[0:64, :])
                nc.scalar.copy(gath[64:96, off:off + n], ps[64:96, :])
                off += n

            # ---------------- likelihood chunk scan ----------------
            lk = big.tile([128, CHUNK], f32)
            for s in range(NSLOT):
                for q in range(4):
                    k = 4 * s + q
                    nc.sync.dma_start(
                        lk[8 * k:8 * k + 8, :],
                        gath[24 * s + 16:24 * s + 24,
                             CHUNK * q:CHUNK * (q + 1)])
            lcum = big.tile([128, CHUNK], f32)
            nc.vector.tensor_tensor_scan(lcum[:], lk[:], lk[:], 0.0,
                                         op0=mybir.AluOpType.add,
                                         op1=mybir.AluOpType.bypass)
            nc.sync.dma_start(lc_out[:], lcum[:])

            # ---------------- per-node outputs ----------------
            for s in range(NSLOT):
                nc.sync.dma_start(
                    hv_out[s].rearrange("f j -> j f"),
                    gath[24 * s:24 * s + 8, :])
                nc.sync.dma_start(
                    hi_out[s].rearrange("f j -> j f"),
                    gath[24 * s + 8:24 * s + 16, :])

    nc.finalize()
    return nc


def _get_nc():
    if "nc" not in _cache:
        _cache["nc"] = _build_nc()
    return _cache["nc"]


def kernel(B, Pi, x, batch, num_graphs):
    B = np.asarray(B, dtype=np.float32)
    Pi = np.asarray(Pi, dtype=np.float32)
    x = np.asarray(x)
    batch = np.asarray(batch)
    ng = int(num_graphs)
    assert B.shape == (C, M, NGEN) and Pi.shape == (C, NGEN)
    assert x.shape == (N,) and batch.shape == (N,) and ng == G

    nc = _get_nc()

    # host-side input prep
    Bp = B.reshape(C, M * NGEN)
    ilo = (np.arange(128) % 32).astype(np.float32).reshape(128, 1)
    ihi = ilo + 32.0
    iotac = np.tile((np.arange(32, dtype=np.float32) + BIG)[None, :],
                    (C, 16)).astype(np.float32)

    x_i = x.astype(np.int64)
    in_maps = []
    for c in range(NCORES):
        xc = x_i[c * NPC:(c + 1) * NPC]
        xc = np.concatenate([xc, np.zeros(NSLOT * SLOT - NPC, np.int64)])
        slots = xc.reshape(NSLOT, 1, SLOT).astype(ml_dtypes.bfloat16)
        xb = np.broadcast_to(slots, (NSLOT, 32, SLOT)).reshape(128, SLOT)
        in_maps.append({
            "Bp": Bp, "Pip": Pi, "xb": np.ascontiguousarray(xb),
            "ilo": ilo, "ihi": ihi, "iotac": iotac,
        })

    res = run_bass_kernel_spmd(nc, in_maps, list(range(NCORES))).results

    # ---------------- host-side unshard / stitch ----------------
    hv = np.empty((N, NGEN), np.float32)
    hi = np.empty((N, NGEN), np.float32)
    lcum = np.empty((NCORES, NCHUNK, NGEN, CHUNK), np.float64)
    for c in range(NCORES):
        r = res[c]
        hv_c = r["hv"].reshape(NSLOT * SLOT, NGEN)[:NPC]
        hi_c = r["hi"].reshape(NSLOT * SLOT, NGEN)[:NPC]
        hv[c * NPC:(c + 1) * NPC] = hv_c
        hi[c * NPC:(c + 1) * NPC] = hi_c
        lcum[c] = r["lc"].reshape(NCHUNK, NGEN, CHUNK)

    # prefix offsets: chunk k of core c covers nodes [2344k, 2344(k+1)) of
    # that core; last real node of core = local 37499 (chunk 15, f 2339).
    chunk_tot = np.empty((NCORES, NCHUNK, NGEN), np.float64)
    chunk_tot[:, :15] = lcum[:, :15, :, CHUNK - 1]
    chunk_tot[:, 15] = lcum[:, 15, :, NPC - 15 * CHUNK - 1]
    flat_tot = chunk_tot.reshape(NCORES * NCHUNK, NGEN)
    chunk_prefix = np.concatenate(
        [np.zeros((1, NGEN)), np.cumsum(flat_tot, axis=0)[:-1]], axis=0
    ).reshape(NCORES, NCHUNK, NGEN)

    # P(p) = sum of lik over global nodes < p, for boundary positions p
    bounds = np.searchsorted(batch, np.arange(G), side="right")  # (G,)

    def prefix_at2(p):
        out = np.zeros((len(p), NGEN), np.float64)
        nz = p > 0
        i = p[nz] - 1
        cc = i // NPC
        rr = i % NPC
        kk = rr // CHUNK
        ff = rr % CHUNK
        vals = lcum[cc, kk, :, ff]  # (n, NGEN)
        out[nz] = chunk_prefix[cc, kk] + vals
        return out

    Pb = prefix_at2(bounds)
    Pb_prev = np.concatenate([np.zeros((1, NGEN)), Pb[:-1]], axis=0)
    likelihood = (Pb - Pb_prev).astype(np.float32)

    h_vals = hv.reshape(N, 1, NGEN)
    h_idx = np.rint(hi).astype(np.int32)
    return likelihood, h_vals, h_idx


# revision 8
# speedup vs baseline: 1.0389x; 1.0389x over previous
"""CGMM layer (segment_reduce) Trainium2 kernel.

Math: every per-node quantity depends on the node only through its discrete
label x_n (64 values), so the kernel reduces to:
  1. a tiny 64x24 table build from B/Pi on device (softmaxes, posterior,
     max/argmax over C via DVE 32x32 block transposes, per-label likelihood),
  2. a table gather for all N nodes done on the tensor engine: one-hot(x)
     built by DVE is_equal against a per-partition label column (x is fed
     pre-replicated across 32 partitions, 4 node-slots per column), then
     block-diagonal matmuls; fp32 table precision is kept with a bf16 hi/lo
     weight split accumulated in PSUM,
  3. an unsegmented prefix sum (DVE tensor_tensor_scan) of the per-node
     likelihood; the host samples it at graph boundaries (known from the
     sorted `batch`) and differences to get per-graph segment sums.

Sharding: data-parallel over nodes, 37500 nodes per core on 8 cores; B/Pi
replicated; outputs disjoint per core except graph sums, stitched on host.

Output column layout of the gather matmul (96 rows): col = 4*j + s for
output j (0..7 h_vals, 8..15 h_idx, 16..23 lik) and node-slot s (0..3),
so h_vals rows = [0:32), h_idx = [32:64), lik = [64:96) - contiguous blocks.
"""

import numpy as np
import ml_dtypes
from contextlib import ExitStack

import concourse.bass as bass
import concourse.tile as tile
import concourse.mybir as mybir
from concourse import bacc
from concourse.bass_utils import run_bass_kernel_spmd

N = 300000
C = 32
M = 64
NGEN = 8
G = 8192
NCORES = 8
NPC = N // NCORES          # 37500 nodes per core
SLOT = 9376                # nodes per slot; 4 slots/core, last 4 nodes pad
NSLOT = 4
BIG = 10000.0
NMM = 512                  # matmul free-dim chunk

_cache = {}


def _build_nc():
    nc = bacc.Bacc("TRN2", target_bir_lowering=False, debug=False,
                   num_devices=NCORES)
    f32, bf16 = mybir.dt.float32, mybir.dt.bfloat16

    Bp = nc.declare_dram_parameter("Bp", [C, M * NGEN], f32, isOutput=False)
    Pip = nc.declare_dram_parameter("Pip", [C, NGEN], f32, isOutput=False)
    xb_in = nc.declare_dram_parameter("xb", [128, SLOT], bf16, isOutput=False)
    ilo_in = nc.declare_dram_parameter("ilo", [128, 1], f32, isOutput=False)
    ihi_in = nc.declare_dram_parameter("ihi", [128, 1], f32, isOutput=False)
    iotac_in = nc.declare_dram_parameter("iotac", [C, M * NGEN], f32, isOutput=False)

    hvhi_out = nc.declare_dram_parameter("hvhi", [64, SLOT], f32, isOutput=True)
    lc_out = nc.declare_dram_parameter("lc", [32, SLOT], f32, isOutput=True)

    with tile.TileContext(nc, num_cores=NCORES) as tc:
        with ExitStack() as ctx:
            small = ctx.enter_context(tc.tile_pool(name="small", bufs=1))
            big = ctx.enter_context(tc.tile_pool(name="big", bufs=1))
            psum = ctx.enter_context(tc.tile_pool(name="ps", bufs=4, space="PSUM"))
            dpool = ctx.enter_context(tc.tile_pool(name="dscratch", bufs=1, space="DRAM"))
            dW = dpool.tile([M, 24], f32)

            # ---------------- inputs ----------------
            xb = big.tile([128, SLOT], bf16)
            nc.sync.dma_start(xb[:], xb_in[:])
            ilo = small.tile([128, 1], f32)
            nc.sync.dma_start(ilo[:], ilo_in[:])
            ihi = small.tile([128, 1], f32)
            nc.sync.dma_start(ihi[:], ihi_in[:])
            iotac = small.tile([C, 512], f32)
            nc.scalar.dma_start(iotac[:], iotac_in[:])
            Bt = small.tile([C, 512], f32)
            nc.scalar.dma_start(Bt[:], Bp[:])
            Pit = small.tile([C, NGEN], f32)
            nc.scalar.dma_start(Pit[:], Pip[:])

            # ---------------- table build (tiny) ----------------
            eB = small.tile([C, 512], f32)
            nc.scalar.activation(eB[:], Bt[:], mybir.ActivationFunctionType.Exp)
            sB = small.tile([C, NGEN], f32)
            nc.vector.tensor_reduce(
                sB[:], eB[:].rearrange("p (m g) -> p g m", g=NGEN),
                axis=mybir.AxisListType.X, op=mybir.AluOpType.add)
            rB = small.tile([C, NGEN], f32)
            nc.vector.reciprocal(rB[:], sB[:])
            smB = small.tile([C, 512], f32)
            nc.vector.tensor_tensor(
                smB[:].rearrange("p (m g) -> p m g", g=NGEN),
                eB[:].rearrange("p (m g) -> p m g", g=NGEN),
                rB[:].unsqueeze(1).broadcast_to((C, M, NGEN)),
                op=mybir.AluOpType.mult)

            ePi = small.tile([C, NGEN], f32)
            nc.scalar.activation(ePi[:], Pit[:], mybir.ActivationFunctionType.Exp)
            piP = small.tile([C, 32], f32)
            nc.vector.memset(piP[:], 0.0)
            nc.vector.tensor_copy(piP[:, 0:NGEN], ePi[:])
            piT = small.tile([C, 32], f32)
            nc.vector.transpose(piT[:], piP[:])          # piT[g, c]
            sPi = small.tile([C, 1], f32)
            nc.vector.tensor_reduce(sPi[:], piT[:], axis=mybir.AxisListType.X,
                                    op=mybir.AluOpType.add)
            rPi = small.tile([C, 1], f32)
            nc.vector.reciprocal(rPi[:], sPi[:])
            smPiT = small.tile([C, 32], f32)
            nc.vector.tensor_scalar(smPiT[:], piT[:], rPi[:, 0:1], None,
                                    op0=mybir.AluOpType.mult)
            smPi32 = small.tile([C, 32], f32)
            nc.vector.transpose(smPi32[:], smPiT[:])     # smPi32[c, g]

            num = small.tile([C, 512], f32)
            nc.vector.tensor_tensor(
                num[:].rearrange("p (m g) -> p m g", g=NGEN),
                smB[:].rearrange("p (m g) -> p m g", g=NGEN),
                smPi32[:, 0:NGEN].unsqueeze(1).broadcast_to((C, M, NGEN)),
                op=mybir.AluOpType.mult)

            # transpose to (mg-within-block, c) for the C-axis reductions
            numT = small.tile([C, 512], f32)
            nc.vector.transpose(numT[:], num[:])
            den = small.tile([C, 16], f32)
            nc.vector.tensor_reduce(
                den[:], numT[:].rearrange("p (k q) -> p k q", q=32),
                axis=mybir.AxisListType.X, op=mybir.AluOpType.add)
            rden = small.tile([C, 16], f32)
            nc.vector.reciprocal(rden[:], den[:])
            postT = small.tile([C, 512], f32)
            nc.vector.tensor_tensor(
                postT[:].rearrange("p (k q) -> p k q", q=32),
                numT[:].rearrange("p (k q) -> p k q", q=32),
                rden[:].unsqueeze(2).broadcast_to((C, 16, 32)),
                op=mybir.AluOpType.mult)
            lognT = small.tile([C, 512], f32)
            nc.scalar.activation(lognT[:], numT[:], mybir.ActivationFunctionType.Ln)
            plT = small.tile([C, 512], f32)
            nc.vector.tensor_tensor(plT[:], postT[:], lognT[:],
                                    op=mybir.AluOpType.mult)
            likmg = small.tile([C, 16], f32)
            nc.vector.tensor_reduce(
                likmg[:], plT[:].rearrange("p (k q) -> p k q", q=32),
                axis=mybir.AxisListType.X, op=mybir.AluOpType.add)
            hvmg = small.tile([C, 16], f32)
            nc.vector.tensor_reduce(
                hvmg[:], postT[:].rearrange("p (k q) -> p k q", q=32),
                axis=mybir.AxisListType.X, op=mybir.AluOpType.max)
            mask = small.tile([C, 512], f32)
            nc.vector.tensor_tensor(
                mask[:].rearrange("p (k q) -> p k q", q=32),
                postT[:].rearrange("p (k q) -> p k q", q=32),
                hvmg[:].unsqueeze(2).broadcast_to((C, 16, 32)),
                op=mybir.AluOpType.is_equal)
            cand = small.tile([C, 512], f32)
            nc.vector.tensor_scalar(cand[:], mask[:], -BIG, None,
                                    op0=mybir.AluOpType.mult)
            cand2 = small.tile([C, 512], f32)
            nc.vector.tensor_tensor(cand2[:], cand[:], iotac[:],
                                    op=mybir.AluOpType.add)
            himg = small.tile([C, 16], f32)
            nc.vector.tensor_reduce(
                himg[:], cand2[:].rearrange("p (k q) -> p k q", q=32),
                axis=mybir.AxisListType.X, op=mybir.AluOpType.min)

            # dump tables into dW[m, 8t+g]: tile element (p=8a+b, k) is the
            # value for mg = 32k + p, i.e. m = 4k + a, g = b
            dWv = dW[:, :].rearrange("(k a) (t b) -> a t b k", a=4, b=8)
            for t, tmg in enumerate((hvmg, himg, likmg)):
                eng = nc.sync if t % 2 == 0 else nc.scalar
                for a in range(4):
                    eng.dma_start(dWv[a, t], tmg[8 * a:8 * a + 8, :])

            # fp32 block-diagonal weights, then bf16 hi/lo split.
            # W[32s + m, 4j + s] = T[m, j]
            Wlo = small.tile([128, 96], f32)
            nc.vector.memset(Wlo[:], 0.0)
            Whi = small.tile([128, 96], f32)
            nc.vector.memset(Whi[:], 0.0)
            for s in range(NSLOT):
                dst = slice(32 * s, 32 * s + 32)
                wv_lo = Wlo[dst, :].rearrange("m (j four) -> m four j", four=4)
                wv_hi = Whi[dst, :].rearrange("m (j four) -> m four j", four=4)
                nc.sync.dma_start(wv_lo[:, s], dW[0:32, :])
                nc.scalar.dma_start(wv_hi[:, s], dW[32:64, :])
            W1 = {}
            W2 = {}
            for name, Wf in (("lo", Wlo), ("hi", Whi)):
                w1 = small.tile([128, 96], bf16, tag=f"w1{name}")
                nc.vector.tensor_copy(w1[:], Wf[:])
                w1f = small.tile([128, 96], f32, tag=f"w1f{name}")
                nc.vector.tensor_copy(w1f[:], w1[:])
                w2f = small.tile([128, 96], f32, tag=f"w2f{name}")
                nc.vector.tensor_tensor(w2f[:], Wf[:], w1f[:],
                                        op=mybir.AluOpType.subtract)
                w2 = small.tile([128, 96], bf16, tag=f"w2{name}")
                nc.vector.tensor_copy(w2[:], w2f[:])
                W1[name], W2[name] = w1, w2

            # ---------------- one-hot + gather + scan ----------------
            oh_lo = big.tile([128, SLOT], bf16)
            nc.vector.tensor_scalar(oh_lo[:], xb[:], ilo[:, 0:1], None,
                                    op0=mybir.AluOpType.is_equal)
            oh_hi = big.tile([128, SLOT], bf16)
            nc.vector.tensor_scalar(oh_hi[:], xb[:], ihi[:, 0:1], None,
                                    op0=mybir.AluOpType.is_equal)

            gath = big.tile([96, SLOT], f32)
            # scan operands must share a base partition; rows [64:96) of this
            # tile line up with the lik rows of `gath`
            lcv = big.tile([96, SLOT], f32)
            lcum = lcv[64:96]
            off = 0
            ci = 0
            while off < SLOT:
                n = min(NMM, SLOT - off)
                sl = slice(off, off + n)
                ps = psum.tile([96, n], f32)
                nc.tensor.matmul(ps[:], W1["lo"], oh_lo[:, sl], start=True, stop=False)
                nc.tensor.matmul(ps[:], W1["hi"], oh_hi[:, sl], start=False, stop=False)
                nc.tensor.matmul(ps[:], W2["lo"], oh_lo[:, sl], start=False, stop=False)
                nc.tensor.matmul(ps[:], W2["hi"], oh_hi[:, sl], start=False, stop=True)
                # evacuate PSUM: hv+hi rows by ACT, lik rows by DVE (into gath)
                nc.scalar.copy(gath[0:64, sl], ps[0:64, :])
                nc.vector.tensor_copy(gath[64:96, sl], ps[64:96, :])
                # chained prefix scan of lik rows
                init = 0.0 if off == 0 else lcum[:, off - 1:off]
                nc.vector.tensor_tensor_scan(
                    lcum[:, sl], gath[64:96, sl], gath[64:96, sl], init,
                    op0=mybir.AluOpType.add, op1=mybir.AluOpType.bypass)
                off += n
                ci += 1

            nc.scalar.dma_start(hvhi_out[:], gath[0:64, :])
            nc.sync.dma_start(lc_out[:], lcum[:])

    nc.finalize()
    return nc


def _get_nc():
    if "nc" not in _cache:
        _cache["nc"] = _build_nc()
    return _cache["nc"]


def _host_prep(B, Pi, x):
    Bp = np.ascontiguousarray(B.reshape(C, M * NGEN))
    ilo = (np.arange(128) % 32).astype(np.float32).reshape(128, 1)
    ihi = ilo + 32.0
    iotac = np.tile((np.arange(32, dtype=np.float32) + BIG)[None, :],
                    (C, 16)).astype(np.float32)
    x_i = x.astype(np.int64)
    in_maps = []
    for c in range(NCORES):
        xc = x_i[c * NPC:(c + 1) * NPC]
        xc = np.concatenate([xc, np.zeros(NSLOT * SLOT - NPC, np.int64)])
        slots = xc.reshape(NSLOT, 1, SLOT).astype(ml_dtypes.bfloat16)
        xb = np.broadcast_to(slots, (NSLOT, 32, SLOT)).reshape(128, SLOT)
        in_maps.append({
            "Bp": Bp, "Pip": Pi, "xb": np.ascontiguousarray(xb),
            "ilo": ilo, "ihi": ihi, "iotac": iotac,
        })
    return in_maps


def _host_post(res, batch):
    hv = np.empty((N, NGEN), np.float32)
    hi = np.empty((N, NGEN), np.float32)
    lcum = np.empty((NCORES, NGEN, NSLOT, SLOT), np.float64)
    for c in range(NCORES):
        hvhi = res[c]["hvhi"]            # [64, SLOT], row = 4j + s
        hv_c = hvhi[0:32].reshape(NGEN, NSLOT, SLOT)
        hi_c = hvhi[32:64].reshape(NGEN, NSLOT, SLOT)
        # node local = SLOT*s + f
        hv[c * NPC:(c + 1) * NPC] = hv_c.transpose(1, 2, 0).reshape(-1, NGEN)[:NPC]
        hi[c * NPC:(c + 1) * NPC] = hi_c.transpose(1, 2, 0).reshape(-1, NGEN)[:NPC]
        lcum[c] = res[c]["lc"].reshape(NGEN, NSLOT, SLOT)

    # chunk (= slot) totals at last real node of each slot
    last_f = [SLOT - 1, SLOT - 1, SLOT - 1, NPC - 3 * SLOT - 1]
    tot = np.stack([lcum[:, :, s, last_f[s]] for s in range(NSLOT)],
                   axis=2)                      # (cores, NGEN, NSLOT)
    flat = tot.transpose(0, 2, 1).reshape(NCORES * NSLOT, NGEN)
    prefix = np.concatenate([np.zeros((1, NGEN)), np.cumsum(flat, 0)[:-1]],
                            0).reshape(NCORES, NSLOT, NGEN)

    bounds = np.searchsorted(batch, np.arange(G), side="right")

    def prefix_at(p):
        out = np.zeros((len(p), NGEN), np.float64)
        nz = p > 0
        i = p[nz] - 1
        cc = i // NPC
        rr = i % NPC
        ss = rr // SLOT
        ff = rr % SLOT
        # advanced indices separated by the ":" slice -> shape (n, NGEN)
        out[nz] = prefix[cc, ss] + lcum[cc, :, ss, ff]
        return out

    Pb = prefix_at(bounds)
    Pb_prev = np.concatenate([np.zeros((1, NGEN)), Pb[:-1]], axis=0)
    likelihood = (Pb - Pb_prev).astype(np.float32)
    h_vals = hv.reshape(N, 1, NGEN)
    h_idx = np.rint(hi).astype(np.int32)
    return likelihood, h_vals, h_idx


def kernel(B, Pi, x, batch, num_graphs):
    B = np.asarray(B, dtype=np.float32)
    Pi = np.asarray(Pi, dtype=np.float32)
    x = np.asarray(x)
    batch = np.asarray(batch)
    assert B.shape == (C, M, NGEN) and Pi.shape == (C, NGEN)
    assert x.shape == (N,) and batch.shape == (N,) and int(num_graphs) == G

    nc = _get_nc()
    in_maps = _host_prep(B, Pi, x)
    res = run_bass_kernel_spmd(nc, in_maps, list(range(NCORES))).results
    return _host_post(res, batch)
